# revision 33
# baseline (speedup 1.0000x reference)
"""Bass/Tile kernel builder for the 2-layer GAT + MLP-head classifier (v3).

Math trick: with e_ij = s_i + d_j and row-softmax, the s_i term cancels:
  alpha = softmax_j(where(A==0, -1e9, s_i + d_j))
        = A_ij * u_j / sum_j(A_ij * u_j),   u = exp(d)
  Z = alpha @ H = (A @ (u*H)) / (A @ u)
so each GAT layer is one masked matmul A @ [u*H, u] -- the NxN attention
matrix is never materialized.

v3 (fp8 DoubleRow):
- A is 0/1 so fp8e4m3 is exact; host packs each core's A^T shard into the
  DoubleRow k-pair layout [N/256, 128, 2, R] so the PE contracts 256 rows
  per chunk at 2 fp8 MACs/cell/cycle.
- B = [u*H*s, u] quantized to fp8 with a power-of-2 scale s on the uH
  columns (s cancels via 1/s folded into W2); layer-2 needs s2=2^16 to
  clear the fp8 normal range. Verified numerically: rel_err ~7e-3.
- B chunks padded to a 16-byte multiple of columns (80 / 48) to satisfy
  the DoubleRow weight AP step%16 constraint; pad columns are zeroed.
- AllGather payloads are fp8 in the same k-pair chunk layout, so the
  per-chunk stationary loads are single contiguous DMAs.
- A tiles double-buffered across reps (8MB/shard), ag/ar bounce buffers
  parity-alternated, so back-to-back reps pipeline.
"""

import math

import numpy as np
import ml_dtypes

import concourse.bass as bass
import concourse.bacc as bacc
import concourse.mybir as mybir
import concourse.tile as tile

F32 = mybir.dt.float32
F32R = mybir.dt.float32r
BF16 = mybir.dt.bfloat16
FP8 = mybir.dt.float8e4
P = 128

DIN = 768
DH1, DO1 = 64, 64
DH2, DO2 = 32, 64
M1P, M2P = 80, 48            # padded B cols (16B multiple for DoubleRow)
S1, S2 = 8.0, 2.0 ** 16      # fp8 uH column scales, folded into W21/W22


def _bf16(x):
    return np.asarray(x, dtype=np.float32).astype(ml_dtypes.bfloat16)


def _f32(x):
    return np.ascontiguousarray(np.asarray(x, dtype=np.float32))


def _f32r_view(h):
    """Reinterpret an f32 DRAM handle as f32r (same bytes) so the DMA into
    an f32r SBUF tile is a pure byte copy, not a cast."""
    from concourse.bass import DRamTensorHandle
    return DRamTensorHandle(h.name, list(h.shape), F32R)


def _halves(R, step=512):
    out = []
    s = 0
    while s < R:
        e = min(s + step, R)
        out.append((s, e))
        s = e
    return out


def build_gat_nc(weights, N=8192, n_cores=8, reps=1, stage=5, debug=False,
                 no_coll=False, core_id=0):
    R = N // n_cores            # rows per core
    C2 = N // 256               # DoubleRow j-chunks (256-deep contraction)
    RC2 = R // 256              # local 256-row groups per core
    H512 = _halves(R, 512)
    DR = mybir.MatmulPerfMode.DoubleRow

    nc = bacc.Bacc("TRN2", debug=debug, num_devices=n_cores,
                   target_bir_lowering=False)
    groups = [list(range(n_cores))]

    # ---------------- I/O ----------------
    xt_d = nc.declare_dram_parameter("XT", [DIN, R], BF16, isOutput=False)
    at_d = nc.declare_dram_parameter("AT", [C2 * P, 2, R], FP8, isOutput=False)
    out_d = nc.declare_dram_parameter("out", [2, 1], F32, isOutput=True)

    # ---------------- constants (inlined into NEFF) ----------------
    w1_d = nc.inline_tensor(_bf16(weights["W1"]), "w1c")           # [768, 64]
    a1d_d = nc.inline_tensor(_bf16(weights["a1"][DH1:]).reshape(DH1, 1), "a1dc")
    w21_d = nc.inline_tensor(_f32(np.asarray(weights["W21"]) / S1), "w21c")
    b21_d = nc.inline_tensor(_f32(weights["b21"]).reshape(DO1, 1), "b21c")
    wg2_d = nc.inline_tensor(_bf16(weights["Wg2"]), "wg2c")        # [64, 32]
    a2d_d = nc.inline_tensor(_bf16(weights["a2"][DH2:]).reshape(DH2, 1), "a2dc")
    w22_d = nc.inline_tensor(_f32(np.asarray(weights["W22"]) / S2), "w22c")
    b22_d = nc.inline_tensor(_f32(weights["b22"]).reshape(DO2, 1), "b22c")
    m1_d = nc.inline_tensor(_f32(weights["M1"]), "m1c")            # [64, 64]
    bm1_d = nc.inline_tensor(_f32(weights["bm1"]).reshape(64, 1), "bm1c")
    m2_d = nc.inline_tensor(_f32(weights["M2"]), "m2c")            # [64, 2]
    bm2_d = nc.inline_tensor(_f32(weights["bm2"]).reshape(2, 1), "bm2c")
    ident_d = nc.inline_tensor(
        np.eye(P, dtype=np.float32).astype(ml_dtypes.bfloat16), "identc")
    ones_d = nc.inline_tensor(np.ones((1, 64), np.float32), "onesc")

    # ---------------- internal DRAM (collective bounce, rep-parity) -------
    ag1_in = [nc.dram_tensor(f"ag1_in{p}", [RC2 * P, 2, M1P], FP8)
              for p in range(2)]
    ag1_out = [nc.dram_tensor(f"ag1_out{p}", [C2 * P, 2, M1P], FP8,
                              addr_space="Shared") for p in range(2)]
    ag2_in = [nc.dram_tensor(f"ag2_in{p}", [RC2 * P, 2, M2P], FP8)
              for p in range(2)]
    ag2_out = [nc.dram_tensor(f"ag2_out{p}", [C2 * P, 2, M2P], FP8,
                              addr_space="Shared") for p in range(2)]
    ar_in = [nc.dram_tensor(f"ar_in{p}", [64, 1], F32) for p in range(2)]
    ar_out = [nc.dram_tensor(f"ar_out{p}", [n_cores, 64], F32,
                             addr_space="Shared") for p in range(2)]

    with tile.TileContext(nc) as tc:
        with (
            tc.tile_pool(name="const", bufs=1) as cp,
            tc.tile_pool(name="work", bufs=1) as wp,
            tc.tile_pool(name="psum", bufs=1, space="PSUM") as sp_pool,
        ):
            # ---- load constants to SBUF (once)
            w1_sb = cp.tile([P, (DIN // P) * DH1], BF16, name="w1_sb")
            for kc in range(DIN // P):
                nc.sync.dma_start(w1_sb[:, kc * DH1:(kc + 1) * DH1],
                                  w1_d[kc * P:(kc + 1) * P, :])
            a1d_sb = cp.tile([DH1, 1], BF16, name="a1d_sb")
            nc.sync.dma_start(a1d_sb, a1d_d[:])
            a2d_sb = cp.tile([DH2, 1], BF16, name="a2d_sb")
            nc.sync.dma_start(a2d_sb, a2d_d[:])
            w21_sb = cp.tile([DH1, DO1], F32R, name="w21_sb")
            nc.sync.dma_start(w21_sb, _f32r_view(w21_d)[:])
            b21_sb = cp.tile([DO1, 1], F32, name="b21_sb")
            nc.sync.dma_start(b21_sb, b21_d[:])
            wg2_sb = cp.tile([DO1, DH2], BF16, name="wg2_sb")
            nc.sync.dma_start(wg2_sb, wg2_d[:])
            w22_sb = cp.tile([DH2, DO2], F32R, name="w22_sb")
            nc.sync.dma_start(w22_sb, _f32r_view(w22_d)[:])
            b22_sb = cp.tile([DO2, 1], F32, name="b22_sb")
            nc.sync.dma_start(b22_sb, b22_d[:])
            m1_sb = cp.tile([64, 64], F32, name="m1_sb")
            nc.sync.dma_start(m1_sb, m1_d[:])
            bm1_sb = cp.tile([64, 1], F32, name="bm1_sb")
            nc.sync.dma_start(bm1_sb, bm1_d[:])
            m2_sb = cp.tile([64, 2], F32, name="m2_sb")
            nc.sync.dma_start(m2_sb, m2_d[:])
            bm2_sb = cp.tile([2, 1], F32, name="bm2_sb")
            nc.sync.dma_start(bm2_sb, bm2_d[:])
            ident_sb = cp.tile([P, P], BF16, name="ident_sb")
            nc.sync.dma_start(ident_sb, ident_d[:])
            ones_sb = cp.tile([1, 64], F32R, name="ones_sb")
            nc.sync.dma_start(ones_sb, _f32r_view(ones_d)[:])
            lns1_sb = cp.tile([P, 1], F32, name="lns1_sb")
            nc.vector.memset(lns1_sb, float(math.log(S1)))
            lns2_sb = cp.tile([P, 1], F32, name="lns2_sb")
            nc.vector.memset(lns2_sb, float(math.log(S2)))
            ones8_sb = cp.tile([n_cores, 1], F32, name="ones8_sb")
            nc.vector.memset(ones8_sb, 1.0)

            # stage-timing scaffolding: tiny accumulator chain that keeps
            # truncated-stage variants honest (anti-DCE, serializes reps)
            f_acc = cp.tile([1, 1], F32, name="f_acc")
            nc.vector.memset(f_acc, 0.0)

            def consume(src, t):
                g = wp.tile([1, 1], F32, tag="csm", bufs=2, name=f"csm{t}")
                nc.vector.tensor_copy(g, src)
                nc.vector.tensor_tensor(f_acc, g, f_acc,
                                        op=mybir.AluOpType.add)

            def build_own_b(ht_sb, dh, mp, a_sb, lns, ag_in, tag):
                """Write this core's R rows of B = [u*H*s, u, 0pad] (fp8,
                k-pair interleaved chunk layout) to ag_in."""
                for g in range(RC2):
                    bch = wp.tile([P, 2, mp], FP8, tag="bch", bufs=3,
                                  name=f"bch{tag}_{g}")
                    for j in range(2):
                        lc = 2 * g + j
                        hTs = ht_sb[:, lc * P:(lc + 1) * P]
                        ps_tr = sp_pool.tile([P, dh], BF16, tag="sm", bufs=4,
                                             name=f"ps_tr{tag}_{lc}")
                        nc.tensor.transpose(ps_tr, hTs, ident_sb[:dh, :dh])
                        ps_d = sp_pool.tile([P, 1], F32, tag="sm", bufs=4,
                                            name=f"ps_d{tag}_{lc}")
                        nc.tensor.matmul(ps_d, hTs, a_sb, start=True,
                                         stop=True)
                        u = wp.tile([P, 1], F32, tag="u", bufs=4,
                                    name=f"u{tag}_{lc}")
                        nc.scalar.activation(u, ps_d,
                                             mybir.ActivationFunctionType.Exp)
                        us = wp.tile([P, 1], F32, tag="u", bufs=4,
                                     name=f"us{tag}_{lc}")
                        nc.scalar.activation(us, ps_d,
                                             mybir.ActivationFunctionType.Exp,
                                             bias=lns)
                        nc.vector.tensor_scalar_mul(bch[:, j, 0:dh], ps_tr, us)
                        nc.vector.tensor_copy(bch[:, j, dh:dh + 1], u)
                        nc.vector.memset(bch[:, j, dh + 1:mp], 0.0)
                    nc.sync.dma_start(ag_in[g * P:(g + 1) * P, :, :], bch)

            def masked_accum(ps_o, ag_out, mp, abf, tag):
                """ps_o[mp, R] += B_c^T @ A_c over all 256-deep chunks."""
                for c in range(C2):
                    bT = wp.tile([P, 2, mp], FP8, tag="bT", bufs=6,
                                 name=f"bT{tag}_{c}")
                    nc.sync.dma_start(bT, ag_out[c * P:(c + 1) * P, :, :])
                    for h, (s, e) in enumerate(H512):
                        nc.tensor.matmul(ps_o[:, s:e], bT, abf[c][:, :, s:e],
                                         start=(c == 0), stop=(c == C2 - 1),
                                         perf_mode=DR)

            def epilogue(ps_o, dh, do, w2_sb, b2_sb, out_dt, tag):
                """out_t = elu((numer/denom) @ W2 + b2), transposed [do, R].

                f32r matmuls (1 cyc/row at free>=256, ~10-bit mantissa):
                broadcast the raw denominator to [do, R], reciprocal at full
                partition width, multiply into the W2 product."""
                o_t = wp.tile([dh, R], F32R, tag="o_t", bufs=2,
                              name=f"o_t{tag}")
                nc.scalar.copy(o_t, ps_o[0:dh, :])
                den = wp.tile([1, R], F32R, tag="den", bufs=2,
                              name=f"den{tag}")
                nc.scalar.copy(den, ps_o[dh:dh + 1, :])
                bc_sb = wp.tile([do, R], F32, tag="bc", bufs=2,
                                name=f"bc{tag}")
                zt = wp.tile([do, R], F32, tag="ep", bufs=4, name=f"zt{tag}")
                for h, (s, e) in enumerate(H512):
                    ps_bc = sp_pool.tile([do, e - s], F32, tag="sm", bufs=4,
                                         name=f"ps_bc{tag}_{h}")
                    nc.tensor.matmul(ps_bc, ones_sb[:, :do], den[:, s:e],
                                     start=True, stop=True)
                    nc.vector.reciprocal(bc_sb[:, s:e], ps_bc)
                    ps_p = sp_pool.tile([do, e - s], F32, tag="sm", bufs=4,
                                        name=f"ps_p{tag}_{h}")
                    nc.tensor.matmul(ps_p, w2_sb, o_t[0:dh, s:e],
                                     start=True, stop=True)
                    nc.vector.tensor_tensor(zt[:, s:e], ps_p, bc_sb[:, s:e],
                                            op=mybir.AluOpType.mult)
                # ELU: r = relu(zt+b2); en = exp(min(zt+b2, 0)); out = r+en-1
                nm = wp.tile([do, R], F32, tag="ep", bufs=4, name=f"nm{tag}")
                nc.vector.tensor_scalar(nm, zt, b2_sb, 0.0,
                                        op0=mybir.AluOpType.add,
                                        op1=mybir.AluOpType.min)
                en = wp.tile([do, R], F32, tag="ep", bufs=4, name=f"en{tag}")
                nc.scalar.activation(en, nm, mybir.ActivationFunctionType.Exp)
                r = wp.tile([do, R], F32, tag="ep", bufs=4, name=f"r{tag}")
                nc.scalar.activation(r, zt, mybir.ActivationFunctionType.Relu,
                                     bias=b2_sb)
                out_t = wp.tile([do, R], out_dt, tag=f"out_t{out_dt}", bufs=2,
                                name=f"out_t{tag}")
                nc.vector.scalar_tensor_tensor(
                    out_t, in0=r, scalar=-1.0, in1=en,
                    op0=mybir.AluOpType.add, op1=mybir.AluOpType.add)
                return out_t

            for rep in range(reps):
                rr = f"r{rep}"
                pa = rep % 2

                # ---- A^T shard + X^T loads (fresh per rep); interleave the
                # first XT chunks among the A chunks so H1 isn't starved.
                abf = [wp.tile([P, 2, R], FP8, tag="abf", bufs=2 * C2,
                               name=f"abf{rr}_{c}") for c in range(C2)]
                xt = [wp.tile([P, R], BF16, tag="xt", bufs=6,
                              name=f"xt{kc}{rr}") for kc in range(DIN // P)]
                for kc in range(DIN // P):
                    nc.sync.dma_start(xt[kc], xt_d[kc * P:(kc + 1) * P, :])
                for c in range(C2):
                    nc.sync.dma_start(abf[c], at_d[c * P:(c + 1) * P, :, :])
                if stage <= 1:
                    consume(abf[C2 - 1][0:1, 0:1, 0:1], "s1" + rr)
                    continue

                # ---- H1^T = W1^T @ X^T
                ps_h1 = sp_pool.tile([DH1, R], F32, tag="big", bufs=2,
                                     name=f"ps_h1{rr}")
                for kc in range(DIN // P):
                    for h, (s, e) in enumerate(H512):
                        nc.tensor.matmul(ps_h1[:, s:e],
                                         w1_sb[:, kc * DH1:(kc + 1) * DH1],
                                         xt[kc][:, s:e],
                                         start=(kc == 0),
                                         stop=(kc == DIN // P - 1))
                h1t_sb = wp.tile([DH1, R], BF16, tag="ht", bufs=2,
                                 name=f"h1t{rr}")
                nc.scalar.copy(h1t_sb, ps_h1)

                # ---- layer 1: own-B build, gather, masked matmul
                build_own_b(h1t_sb, DH1, M1P, a1d_sb, lns1_sb,
                            ag1_in[pa], "b1" + rr)
                if no_coll:
                    nc.sync.dma_start(ag1_out[pa][0:RC2 * P, :, :],
                                      ag1_in[pa][:])
                else:
                    nc.gpsimd.collective_compute(
                        "AllGather", mybir.AluOpType.bypass,
                        replica_groups=groups,
                        ins=[ag1_in[pa][:]], outs=[ag1_out[pa][:]])
                if stage <= 2:
                    gt = wp.tile([1, 1], FP8, tag="gt", bufs=2,
                                 name=f"gt{rr}")
                    nc.sync.dma_start(gt, ag1_out[pa][C2 * P - 1:C2 * P,
                                                     1:2, 0:1])
                    consume(gt, "s2" + rr)
                    continue
                ps_o1 = sp_pool.tile([M1P, R], F32, tag="big", bufs=2,
                                     name=f"ps_o1{rr}")
                masked_accum(ps_o1, ag1_out[pa], M1P, abf, "1" + rr)
                out1t = epilogue(ps_o1, DH1, DO1, w21_sb, b21_sb, BF16,
                                 "1" + rr)
                if stage <= 3:
                    consume(out1t[0:1, 0:1], "s3" + rr)
                    continue

                # ---- layer 2
                ps_h2 = sp_pool.tile([DH2, R], F32, tag="big", bufs=2,
                                     name=f"ps_h2{rr}")
                for h, (s, e) in enumerate(H512):
                    nc.tensor.matmul(ps_h2[:, s:e], wg2_sb, out1t[:, s:e],
                                     start=True, stop=True)
                h2t_sb = wp.tile([DH2, R], BF16, tag="ht2", bufs=2,
                                 name=f"h2t{rr}")
                nc.scalar.copy(h2t_sb, ps_h2)

                build_own_b(h2t_sb, DH2, M2P, a2d_sb, lns2_sb,
                            ag2_in[pa], "b2" + rr)
                if no_coll:
                    nc.sync.dma_start(ag2_out[pa][0:RC2 * P, :, :],
                                      ag2_in[pa][:])
                else:
                    nc.gpsimd.collective_compute(
                        "AllGather", mybir.AluOpType.bypass,
                        replica_groups=groups,
                        ins=[ag2_in[pa][:]], outs=[ag2_out[pa][:]])
                ps_o2 = sp_pool.tile([M2P, R], F32, tag="big", bufs=2,
                                     name=f"ps_o2{rr}")
                masked_accum(ps_o2, ag2_out[pa], M2P, abf, "2" + rr)
                out2t = epilogue(ps_o2, DH2, DO2, w22_sb, b22_sb, F32,
                                 "2" + rr)
                if stage <= 4:
                    consume(out2t[0:1, 0:1], "s4" + rr)
                    continue

                # ---- head: mean over all rows -> MLP -> [2]
                gsum = wp.tile([64, 1], F32, tag="gsum", bufs=4,
                               name=f"gsum{rr}")
                nc.vector.reduce_sum(gsum, out2t, axis=mybir.AxisListType.X)
                nc.sync.dma_start(ar_in[pa][:], gsum)
                if no_coll:
                    nc.sync.dma_start(ar_out[pa][0:1, :], ar_in[pa][:])
                else:
                    # single-phase AllGather beats 2-phase AllReduce latency;
                    # sum the 8 per-core partials locally (g8^T @ ones).
                    nc.gpsimd.collective_compute(
                        "AllGather", mybir.AluOpType.bypass,
                        replica_groups=groups,
                        ins=[ar_in[pa][:]], outs=[ar_out[pa][:]])
                g8 = wp.tile([n_cores, 64], F32, tag="gsum", bufs=4,
                             name=f"g8{rr}")
                nc.sync.dma_start(g8, ar_out[pa][:])
                ps_g = sp_pool.tile([64, 1], F32, tag="sm", bufs=4,
                                    name=f"ps_g{rr}")
                nc.tensor.matmul(ps_g, g8, ones8_sb, start=True, stop=True)
                g_sb = wp.tile([64, 1], F32, tag="gsum", bufs=4,
                               name=f"g{rr}")
                nc.scalar.copy(g_sb, ps_g)
                ps_hd = sp_pool.tile([64, 1], F32, tag="sm", bufs=4,
                                     name=f"ps_hd{rr}")
                nc.tensor.matmul(ps_hd, m1_sb, g_sb, start=True, stop=True)
                h_sb = wp.tile([64, 1], F32, tag="gsum", bufs=4,
                               name=f"h{rr}")
                nc.scalar.activation(h_sb, ps_hd,
                                     mybir.ActivationFunctionType.Relu,
                                     bias=bm1_sb, scale=1.0 / N)
                ps_f = sp_pool.tile([2, 1], F32, tag="sm", bufs=4,
                                    name=f"ps_f{rr}")
                nc.tensor.matmul(ps_f, m2_sb, h_sb, start=True, stop=True)
                f_sb = wp.tile([2, 1], F32, tag="gsum", bufs=4,
                               name=f"f{rr}")
                nc.scalar.activation(f_sb, ps_f,
                                     mybir.ActivationFunctionType.Identity,
                                     bias=bm2_sb)
                nc.sync.dma_start(out_d[:], f_sb)

            if stage <= 4:
                nc.sync.dma_start(out_d[0:1, :], f_acc)

    return nc


def numpy_reference(X, A, w):
    """Straight fp32 numpy port of the jax reference (for small-N checks)."""
    def softmax(e):
        m = e.max(axis=1, keepdims=True)
        x = np.exp(e - m)
        return x / x.sum(axis=1, keepdims=True)

    def gat(Xl, W, a, W2, b2):
        H = Xl @ W
        dh = W.shape[1]
        e = (H @ a[:dh])[:, None] + (H @ a[dh:])[None, :]
        e = np.where(A == 0, np.float32(-1e9), e).astype(np.float32)
        alpha = softmax(e)
        Z = alpha @ H
        zz = Z @ W2 + b2
        return np.where(zz > 0, zz, np.exp(np.minimum(zz, 0)) - 1)

    Z = gat(X, w["W1"], w["a1"], w["W21"], w["b21"])
    Z = gat(Z, A, w["Wg2"], w["a2"], w["W22"], w["b22"])
    g = Z.mean(axis=0)
    h = np.maximum(g @ w["M1"] + w["bm1"], 0)
    return h @ w["M2"] + w["bm2"]


def make_in_maps(X, A, N, n_cores):
    """Host-side sharding: XT bf16 [768, R]; AT fp8 (exact for 0/1) packed
    into the DoubleRow k-pair chunk layout [C2*128, 2, R]."""
    R = N // n_cores
    C2 = N // 256
    Xb = np.asarray(X, np.float32).astype(ml_dtypes.bfloat16)
    Af = np.asarray(A, np.float32).astype(ml_dtypes.float8_e4m3)
    maps = []
    for k in range(n_cores):
        At = Af[k * R:(k + 1) * R, :].T                  # [N, R]
        Ap = np.ascontiguousarray(
            At.reshape(C2, 2, P, R).transpose(0, 2, 1, 3)
        ).reshape(C2 * P, 2, R)
        maps.append({
            "XT": np.ascontiguousarray(Xb[k * R:(k + 1) * R, :].T),
            "AT": Ap,
        })
    return maps


# =====================================================================
# Harness entry point: full inputs in, full output out.
# =====================================================================

_KERNEL_STATE = {}


def kernel(**inputs):
    import numpy as np
    from concourse.bass_utils import run_bass_kernel_spmd

    N = inputs["A"].shape[0]
    n_cores = 8
    w = {k: np.asarray(inputs[k]) for k in
         ("W1", "a1", "W21", "b21", "Wg2", "a2", "W22", "b22",
          "M1", "bm1", "M2", "bm2")}
    nc = build_gat_nc(w, N=N, n_cores=n_cores, reps=1, debug=False)
    nc.finalize()
    in_maps = make_in_maps(np.asarray(inputs["X"]), np.asarray(inputs["A"]),
                           N, n_cores)
    res = run_bass_kernel_spmd(nc, in_maps, core_ids=list(range(n_cores)),
                               trace=False)
    _KERNEL_STATE.update(results=res, nc=nc, in_maps=in_maps, w=w, N=N)
    return np.asarray(res.results[0]["out"]).reshape(2).astype(np.float32)


# revision 45
# speedup vs baseline: 1.3617x; 1.3617x over previous
"""Bass/Tile kernel builder for the 2-layer GAT + MLP-head classifier (v3).

Math trick: with e_ij = s_i + d_j and row-softmax, the s_i term cancels:
  alpha = softmax_j(where(A==0, -1e9, s_i + d_j))
        = A_ij * u_j / sum_j(A_ij * u_j),   u = exp(d)
  Z = alpha @ H = (A @ (u*H)) / (A @ u)
so each GAT layer is one masked matmul A @ [u*H, u] -- the NxN attention
matrix is never materialized.

v3 (fp8 DoubleRow):
- A is 0/1 so fp8e4m3 is exact; host packs each core's A^T shard into the
  DoubleRow k-pair layout [N/256, 128, 2, R] so the PE contracts 256 rows
  per chunk at 2 fp8 MACs/cell/cycle.
- B = [u*H*s, u] quantized to fp8 with a power-of-2 scale s on the uH
  columns (s cancels via 1/s folded into W2); layer-2 needs s2=2^16 to
  clear the fp8 normal range. Verified numerically: rel_err ~7e-3.
- B chunks padded to a 16-byte multiple of columns (80 / 48) to satisfy
  the DoubleRow weight AP step%16 constraint; pad columns are zeroed.
- AllGather payloads are fp8 in the same k-pair chunk layout, so the
  per-chunk stationary loads are single contiguous DMAs.
- A tiles double-buffered across reps (8MB/shard), ag/ar bounce buffers
  parity-alternated, so back-to-back reps pipeline.
"""

import math

import numpy as np
import ml_dtypes

import concourse.bass as bass
import concourse.bacc as bacc
import concourse.mybir as mybir
import concourse.tile as tile

F32 = mybir.dt.float32
F32R = mybir.dt.float32r
BF16 = mybir.dt.bfloat16
FP8 = mybir.dt.float8e4
P = 128

DIN = 768
DH1, DO1 = 64, 64
DH2, DO2 = 32, 64
M1P, M2P = 80, 48            # padded B cols (16B multiple for DoubleRow)
S1, S2 = 8.0, 2.0 ** 16      # fp8 uH column scales, folded into W21/W22


def _bf16(x):
    return np.asarray(x, dtype=np.float32).astype(ml_dtypes.bfloat16)


def _f32(x):
    return np.ascontiguousarray(np.asarray(x, dtype=np.float32))


def _f32r_view(h):
    """Reinterpret an f32 DRAM handle as f32r (same bytes) so the DMA into
    an f32r SBUF tile is a pure byte copy, not a cast."""
    from concourse.bass import DRamTensorHandle
    return DRamTensorHandle(h.name, list(h.shape), F32R)


def _halves(R, step=512):
    out = []
    s = 0
    while s < R:
        e = min(s + step, R)
        out.append((s, e))
        s = e
    return out


def build_gat_nc(weights, N=8192, n_cores=8, reps=1, stage=5, debug=False,
                 no_coll=False, core_id=0):
    R = N // n_cores            # rows per core
    C2 = N // 256               # DoubleRow j-chunks (256-deep contraction)
    RC2 = R // 256              # local 256-row groups per core
    H512 = _halves(R, 512)
    DR = mybir.MatmulPerfMode.DoubleRow

    nc = bacc.Bacc("TRN2", debug=debug, num_devices=n_cores,
                   target_bir_lowering=False)
    groups = [list(range(n_cores))]

    # ---------------- I/O ----------------
    xt_d = nc.declare_dram_parameter("XT", [DIN, R], BF16, isOutput=False)
    at_d = nc.declare_dram_parameter("AT", [C2 * P, 2, R], FP8, isOutput=False)
    out_d = nc.declare_dram_parameter("out", [2, 1], F32, isOutput=True)

    # ---------------- constants (inlined into NEFF) ----------------
    w1_d = nc.inline_tensor(_bf16(weights["W1"]), "w1c")           # [768, 64]
    a1d_d = nc.inline_tensor(_bf16(weights["a1"][DH1:]).reshape(DH1, 1), "a1dc")
    w21_d = nc.inline_tensor(_f32(np.asarray(weights["W21"]) / S1), "w21c")
    b21_d = nc.inline_tensor(_f32(weights["b21"]).reshape(DO1, 1), "b21c")
    wg2_d = nc.inline_tensor(_bf16(weights["Wg2"]), "wg2c")        # [64, 32]
    a2d_d = nc.inline_tensor(_bf16(weights["a2"][DH2:]).reshape(DH2, 1), "a2dc")
    w22_d = nc.inline_tensor(_f32(np.asarray(weights["W22"]) / S2), "w22c")
    b22_d = nc.inline_tensor(_f32(weights["b22"]).reshape(DO2, 1), "b22c")
    m1_d = nc.inline_tensor(_f32(weights["M1"]), "m1c")            # [64, 64]
    bm1_d = nc.inline_tensor(_f32(weights["bm1"]).reshape(64, 1), "bm1c")
    m2_d = nc.inline_tensor(_f32(weights["M2"]), "m2c")            # [64, 2]
    bm2_d = nc.inline_tensor(_f32(weights["bm2"]).reshape(2, 1), "bm2c")
    ident_d = nc.inline_tensor(
        np.eye(P, dtype=np.float32).astype(ml_dtypes.bfloat16), "identc")
    ones_d = nc.inline_tensor(np.ones((1, 64), np.float32), "onesc")

    # ---------------- internal DRAM (collective bounce, rep-parity) -------
    ag1_in = [nc.dram_tensor(f"ag1_in{p}", [RC2 * P, 2, M1P], FP8)
              for p in range(2)]
    ag1_out = [nc.dram_tensor(f"ag1_out{p}", [C2 * P, 2, M1P], FP8,
                              addr_space="Shared") for p in range(2)]
    ag2_in = [nc.dram_tensor(f"ag2_in{p}", [RC2 * P, 2, M2P], FP8)
              for p in range(2)]
    ag2_out = [nc.dram_tensor(f"ag2_out{p}", [C2 * P, 2, M2P], FP8,
                              addr_space="Shared") for p in range(2)]
    ar_in = [nc.dram_tensor(f"ar_in{p}", [64, 1], F32) for p in range(2)]
    ar_out = [nc.dram_tensor(f"ar_out{p}", [n_cores, 64], F32,
                             addr_space="Shared") for p in range(2)]

    with tile.TileContext(nc) as tc:
        with (
            tc.tile_pool(name="const", bufs=1) as cp,
            tc.tile_pool(name="work", bufs=1) as wp,
            tc.tile_pool(name="psum", bufs=1, space="PSUM") as sp_pool,
        ):
            # ---- load constants to SBUF (once)
            w1_sb = cp.tile([P, (DIN // P) * DH1], BF16, name="w1_sb")
            for kc in range(DIN // P):
                nc.sync.dma_start(w1_sb[:, kc * DH1:(kc + 1) * DH1],
                                  w1_d[kc * P:(kc + 1) * P, :])
            a1d_sb = cp.tile([DH1, 1], BF16, name="a1d_sb")
            nc.sync.dma_start(a1d_sb, a1d_d[:])
            a2d_sb = cp.tile([DH2, 1], BF16, name="a2d_sb")
            nc.sync.dma_start(a2d_sb, a2d_d[:])
            w21_sb = cp.tile([DH1, DO1], F32R, name="w21_sb")
            nc.sync.dma_start(w21_sb, _f32r_view(w21_d)[:])
            b21_sb = cp.tile([DO1, 1], F32, name="b21_sb")
            nc.sync.dma_start(b21_sb, b21_d[:])
            wg2_sb = cp.tile([DO1, DH2], BF16, name="wg2_sb")
            nc.sync.dma_start(wg2_sb, wg2_d[:])
            w22_sb = cp.tile([DH2, DO2], F32R, name="w22_sb")
            nc.sync.dma_start(w22_sb, _f32r_view(w22_d)[:])
            b22_sb = cp.tile([DO2, 1], F32, name="b22_sb")
            nc.sync.dma_start(b22_sb, b22_d[:])
            m1_sb = cp.tile([64, 64], F32, name="m1_sb")
            nc.sync.dma_start(m1_sb, m1_d[:])
            bm1_sb = cp.tile([64, 1], F32, name="bm1_sb")
            nc.sync.dma_start(bm1_sb, bm1_d[:])
            m2_sb = cp.tile([64, 2], F32, name="m2_sb")
            nc.sync.dma_start(m2_sb, m2_d[:])
            bm2_sb = cp.tile([2, 1], F32, name="bm2_sb")
            nc.sync.dma_start(bm2_sb, bm2_d[:])
            ident_sb = cp.tile([P, P], BF16, name="ident_sb")
            nc.sync.dma_start(ident_sb, ident_d[:])
            ones_sb = cp.tile([1, 64], F32R, name="ones_sb")
            nc.sync.dma_start(ones_sb, _f32r_view(ones_d)[:])
            lns1_sb = cp.tile([P, 1], F32, name="lns1_sb")
            nc.vector.memset(lns1_sb, float(math.log(S1)))
            lns2_sb = cp.tile([P, 1], F32, name="lns2_sb")
            nc.vector.memset(lns2_sb, float(math.log(S2)))
            ones8_sb = cp.tile([n_cores, 1], F32, name="ones8_sb")
            nc.vector.memset(ones8_sb, 1.0)

            # stage-timing scaffolding: tiny accumulator chain that keeps
            # truncated-stage variants honest (anti-DCE, serializes reps)
            f_acc = cp.tile([1, 1], F32, name="f_acc")
            nc.vector.memset(f_acc, 0.0)

            def consume(src, t):
                g = wp.tile([1, 1], F32, tag="csm", bufs=2, name=f"csm{t}")
                nc.vector.tensor_copy(g, src)
                nc.vector.tensor_tensor(f_acc, g, f_acc,
                                        op=mybir.AluOpType.add)

            def build_own_b(ht_sb, dh, mp, a_sb, lns, ag_in, tag):
                """Write this core's R rows of B = [u*H*s, u, 0pad] (fp8,
                k-pair interleaved chunk layout) to ag_in."""
                for g in range(RC2):
                    bch = wp.tile([P, 2, mp], FP8, tag="bch", bufs=3,
                                  name=f"bch{tag}_{g}")
                    for j in range(2):
                        lc = 2 * g + j
                        hTs = ht_sb[:, lc * P:(lc + 1) * P]
                        ps_tr = sp_pool.tile([P, dh], BF16, tag="sm", bufs=4,
                                             name=f"ps_tr{tag}_{lc}")
                        nc.tensor.transpose(ps_tr, hTs, ident_sb[:dh, :dh])
                        ps_d = sp_pool.tile([P, 1], F32, tag="sm", bufs=4,
                                            name=f"ps_d{tag}_{lc}")
                        nc.tensor.matmul(ps_d, hTs, a_sb, start=True,
                                         stop=True)
                        u = wp.tile([P, 1], F32, tag="u", bufs=4,
                                    name=f"u{tag}_{lc}")
                        nc.scalar.activation(u, ps_d,
                                             mybir.ActivationFunctionType.Exp)
                        us = wp.tile([P, 1], F32, tag="u", bufs=4,
                                     name=f"us{tag}_{lc}")
                        nc.scalar.activation(us, ps_d,
                                             mybir.ActivationFunctionType.Exp,
                                             bias=lns)
                        nc.vector.tensor_scalar_mul(bch[:, j, 0:dh], ps_tr, us)
                        nc.vector.tensor_copy(bch[:, j, dh:dh + 1], u)
                        nc.vector.memset(bch[:, j, dh + 1:mp], 0.0)
                    nc.sync.dma_start(ag_in[g * P:(g + 1) * P, :, :], bch)

            def masked_accum(ps_o, ag_out, mp, abf, tag):
                """ps_o[mp, R] += B_c^T @ A_c over all 256-deep chunks."""
                for c in range(C2):
                    bT = wp.tile([P, 2, mp], FP8, tag="bT", bufs=6,
                                 name=f"bT{tag}_{c}")
                    nc.sync.dma_start(bT, ag_out[c * P:(c + 1) * P, :, :])
                    for h, (s, e) in enumerate(H512):
                        nc.tensor.matmul(ps_o[:, s:e], bT, abf[c][:, :, s:e],
                                         start=(c == 0), stop=(c == C2 - 1),
                                         perf_mode=DR)

            def epilogue(ps_o, dh, do, w2_sb, b2_sb, out_dt, tag):
                """out_t = elu((numer/denom) @ W2 + b2), transposed [do, R].

                f32r matmuls (1 cyc/row at free>=256, ~10-bit mantissa):
                broadcast the raw denominator to [do, R], reciprocal at full
                partition width, multiply into the W2 product."""
                o_t = wp.tile([dh, R], F32R, tag="o_t", bufs=2,
                              name=f"o_t{tag}")
                nc.scalar.copy(o_t, ps_o[0:dh, :])
                den = wp.tile([1, R], F32R, tag="den", bufs=1,
                              name=f"den{tag}")
                nc.scalar.copy(den, ps_o[dh:dh + 1, :])
                bc_sb = wp.tile([do, R], F32, tag="bc", bufs=2,
                                name=f"bc{tag}")
                zt = wp.tile([do, R], F32, tag="ep", bufs=4, name=f"zt{tag}")
                for h, (s, e) in enumerate(H512):
                    ps_bc = sp_pool.tile([do, e - s], F32, tag="sm", bufs=4,
                                         name=f"ps_bc{tag}_{h}")
                    nc.tensor.matmul(ps_bc, ones_sb[:, :do], den[:, s:e],
                                     start=True, stop=True)
                    nc.vector.reciprocal(bc_sb[:, s:e], ps_bc)
                    ps_p = sp_pool.tile([do, e - s], F32, tag="sm", bufs=4,
                                        name=f"ps_p{tag}_{h}")
                    nc.tensor.matmul(ps_p, w2_sb, o_t[0:dh, s:e],
                                     start=True, stop=True)
                    nc.vector.tensor_tensor(zt[:, s:e], ps_p, bc_sb[:, s:e],
                                            op=mybir.AluOpType.mult)
                # ELU: r = relu(zt+b2); en = exp(min(zt+b2, 0)); out = r+en-1
                nm = wp.tile([do, R], F32, tag="ep", bufs=4, name=f"nm{tag}")
                nc.vector.tensor_scalar(nm, zt, b2_sb, 0.0,
                                        op0=mybir.AluOpType.add,
                                        op1=mybir.AluOpType.min)
                en = wp.tile([do, R], F32, tag="ep", bufs=4, name=f"en{tag}")
                nc.scalar.activation(en, nm, mybir.ActivationFunctionType.Exp)
                r = wp.tile([do, R], F32, tag="ep", bufs=4, name=f"r{tag}")
                nc.scalar.activation(r, zt, mybir.ActivationFunctionType.Relu,
                                     bias=b2_sb)
                nb = 2 if out_dt == BF16 else 1
                out_t = wp.tile([do, R], out_dt, tag=f"out_t{out_dt}",
                                bufs=nb, name=f"out_t{tag}")
                nc.vector.scalar_tensor_tensor(
                    out_t, in0=r, scalar=-1.0, in1=en,
                    op0=mybir.AluOpType.add, op1=mybir.AluOpType.add)
                return out_t

            # ============================================================
            # Phase functions (i = rep index). State shared via parity dicts.
            # ============================================================
            abf_gen = {}   # parity -> list of 32 A-chunk tiles
            xt_gen = {}    # parity -> list of 6 XT tiles
            ag_coll = (lambda ag_i, ag_o: (
                nc.sync.dma_start(ag_o[0:RC2 * P, :, :], ag_i[:])
                if no_coll else nc.gpsimd.collective_compute(
                    "AllGather", mybir.AluOpType.bypass,
                    replica_groups=groups, ins=[ag_i[:]], outs=[ag_o[:]])))

            def phase_Fdma(i, prefetch_xt=True):
                """Issue rep-i A-shard loads (+ rep-(i+1) XT prefetch) on the
                scalar DMA ring so small loads never queue behind 8MB of A."""
                rr = f"r{i}"
                if prefetch_xt and i + 1 < reps:
                    xt_gen[(i + 1) % 2] = xts = [
                        wp.tile([P, R], BF16, tag="xt", bufs=9,
                                name=f"xt{kc}r{i + 1}") for kc in
                        range(DIN // P)]
                    for kc in range(DIN // P):
                        nc.sync.dma_start(xts[kc],
                                          xt_d[kc * P:(kc + 1) * P, :])
                abf_gen[i % 2] = abf = [
                    wp.tile([P, 2, R], FP8, tag="abf", bufs=2 * C2,
                            name=f"abf{rr}_{c}") for c in range(C2)]
                for c in range(C2):
                    nc.sync.dma_start(abf[c], at_d[c * P:(c + 1) * P, :, :])
                return abf

            def phase_Frest(i):
                """H1 (from prefetched XT), B1 build, AllGather-1 issue."""
                rr = f"r{i}"
                xts = xt_gen[i % 2]
                ps_h1 = sp_pool.tile([DH1, R], F32, tag="big", bufs=2,
                                     name=f"ps_h1{rr}")
                for kc in range(DIN // P):
                    for h, (s, e) in enumerate(H512):
                        nc.tensor.matmul(ps_h1[:, s:e],
                                         w1_sb[:, kc * DH1:(kc + 1) * DH1],
                                         xts[kc][:, s:e],
                                         start=(kc == 0),
                                         stop=(kc == DIN // P - 1))
                h1t_sb = wp.tile([DH1, R], BF16, tag="ht", bufs=2,
                                 name=f"h1t{rr}")
                nc.scalar.copy(h1t_sb, ps_h1)
                build_own_b(h1t_sb, DH1, M1P, a1d_sb, lns1_sb,
                            ag1_in[i % 2], "b1" + rr)
                ag_coll(ag1_in[i % 2], ag1_out[i % 2])

            def phase_Ma(i):
                """Masked matmul layer 1 + epilogue 1 -> out1t."""
                rr = f"r{i}"
                ps_o1 = sp_pool.tile([M1P, R], F32, tag="big", bufs=2,
                                     name=f"ps_o1{rr}")
                masked_accum(ps_o1, ag1_out[i % 2], M1P, abf_gen[i % 2],
                             "1" + rr)
                return epilogue(ps_o1, DH1, DO1, w21_sb, b21_sb, BF16,
                                "1" + rr)

            def phase_Mb(i, out1t):
                """H2, B2 build, AllGather-2 issue."""
                rr = f"r{i}"
                ps_h2 = sp_pool.tile([DH2, R], F32, tag="big", bufs=2,
                                     name=f"ps_h2{rr}")
                for h, (s, e) in enumerate(H512):
                    nc.tensor.matmul(ps_h2[:, s:e], wg2_sb, out1t[:, s:e],
                                     start=True, stop=True)
                h2t_sb = wp.tile([DH2, R], BF16, tag="ht2", bufs=2,
                                 name=f"h2t{rr}")
                nc.scalar.copy(h2t_sb, ps_h2)
                build_own_b(h2t_sb, DH2, M2P, a2d_sb, lns2_sb,
                            ag2_in[i % 2], "b2" + rr)
                ag_coll(ag2_in[i % 2], ag2_out[i % 2])

            def phase_Ta(i):
                """Masked matmul layer 2 + epilogue 2 -> out2t."""
                rr = f"r{i}"
                ps_o2 = sp_pool.tile([M2P, R], F32, tag="big", bufs=2,
                                     name=f"ps_o2{rr}")
                masked_accum(ps_o2, ag2_out[i % 2], M2P, abf_gen[i % 2],
                             "2" + rr)
                return epilogue(ps_o2, DH2, DO2, w22_sb, b22_sb, F32,
                                "2" + rr)

            def phase_Tb(i, out2t):
                """Partial mean + head AllGather issue."""
                rr = f"r{i}"
                gsum = wp.tile([64, 1], F32, tag="gsum", bufs=8,
                               name=f"gsum{rr}")
                nc.vector.reduce_sum(gsum, out2t, axis=mybir.AxisListType.X)
                nc.sync.dma_start(ar_in[i % 2][:], gsum)
                if no_coll:
                    nc.sync.dma_start(ar_out[i % 2][0:1, :], ar_in[i % 2][:])
                else:
                    nc.gpsimd.collective_compute(
                        "AllGather", mybir.AluOpType.bypass,
                        replica_groups=groups,
                        ins=[ar_in[i % 2][:]], outs=[ar_out[i % 2][:]])

            def phase_H(i):
                """Cross-core sum (g8^T @ ones) + MLP head + out."""
                rr = f"r{i}"
                g8 = wp.tile([n_cores, 64], F32, tag="gsum", bufs=8,
                             name=f"g8{rr}")
                nc.sync.dma_start(g8, ar_out[i % 2][:])
                ps_g = sp_pool.tile([64, 1], F32, tag="sm", bufs=4,
                                    name=f"ps_g{rr}")
                nc.tensor.matmul(ps_g, g8, ones8_sb, start=True, stop=True)
                g_sb = wp.tile([64, 1], F32, tag="gsum", bufs=8,
                               name=f"g{rr}")
                nc.scalar.copy(g_sb, ps_g)
                ps_hd = sp_pool.tile([64, 1], F32, tag="sm", bufs=4,
                                     name=f"ps_hd{rr}")
                nc.tensor.matmul(ps_hd, m1_sb, g_sb, start=True, stop=True)
                h_sb = wp.tile([64, 1], F32, tag="gsum", bufs=8,
                               name=f"h{rr}")
                nc.scalar.activation(h_sb, ps_hd,
                                     mybir.ActivationFunctionType.Relu,
                                     bias=bm1_sb, scale=1.0 / N)
                ps_f = sp_pool.tile([2, 1], F32, tag="sm", bufs=4,
                                    name=f"ps_f{rr}")
                nc.tensor.matmul(ps_f, m2_sb, h_sb, start=True, stop=True)
                f_sb = wp.tile([2, 1], F32, tag="gsum", bufs=8,
                               name=f"f{rr}")
                nc.scalar.activation(f_sb, ps_f,
                                     mybir.ActivationFunctionType.Identity,
                                     bias=bm2_sb)
                nc.sync.dma_start(out_d[:], f_sb)

            if stage >= 5:
                # Software-pipelined schedule: iteration i runs
                #   Ma/Mb(i-1) | Ta/Tb(i-2) | H(i-3) | Frest(i) | Fdma(i)
                # so each AllGather gets ~a full iteration of other reps'
                # work to complete before its consumer hits the in-order
                # PE queue. XT is prefetched one iteration ahead; A loads
                # go last (their buffers free up when Ta(i-2) retires).
                xt_gen[0] = xt0 = [
                    wp.tile([P, R], BF16, tag="xt", bufs=9,
                            name=f"xt{kc}r0") for kc in range(DIN // P)]
                for kc in range(DIN // P):
                    nc.sync.dma_start(xt0[kc], xt_d[kc * P:(kc + 1) * P, :])
                for i in range(reps + 3):
                    if 1 <= i <= reps:
                        phase_Mb(i - 1, phase_Ma(i - 1))
                    if 2 <= i <= reps + 1:
                        phase_Tb(i - 2, phase_Ta(i - 2))
                    if 3 <= i <= reps + 2:
                        phase_H(i - 3)
                    if i < reps:
                        phase_Frest(i)
                        phase_Fdma(i)
            else:
                # sequential fallback with stage gates (attribution builds)
                for rep in range(reps):
                    rr = f"r{rep}"
                    xt_gen[rep % 2] = xts = [
                        wp.tile([P, R], BF16, tag="xt", bufs=9,
                                name=f"xt{kc}{rr}") for kc in
                        range(DIN // P)]
                    for kc in range(DIN // P):
                        nc.sync.dma_start(xts[kc],
                                          xt_d[kc * P:(kc + 1) * P, :])
                    abf = phase_Fdma(rep, prefetch_xt=False)
                    if stage <= 1:
                        consume(abf[C2 - 1][0:1, 0:1, 0:1], "s1" + rr)
                        continue
                    phase_Frest(rep)
                    if stage <= 2:
                        gt = wp.tile([1, 1], FP8, tag="gt", bufs=2,
                                     name=f"gt{rr}")
                        nc.sync.dma_start(
                            gt, ag1_out[rep % 2][C2 * P - 1:C2 * P, 1:2, 0:1])
                        consume(gt, "s2" + rr)
                        continue
                    out1t = phase_Ma(rep)
                    if stage <= 3:
                        consume(out1t[0:1, 0:1], "s3" + rr)
                        continue
                    phase_Mb(rep, out1t)
                    out2t = phase_Ta(rep)
                    if stage <= 4:
                        consume(out2t[0:1, 0:1], "s4" + rr)
                        continue
                    phase_Tb(rep, out2t)
                    phase_H(rep)

            if stage <= 4:
                nc.sync.dma_start(out_d[0:1, :], f_acc)

    return nc


def numpy_reference(X, A, w):
    """Straight fp32 numpy port of the jax reference (for small-N checks)."""
    def softmax(e):
        m = e.max(axis=1, keepdims=True)
        x = np.exp(e - m)
        return x / x.sum(axis=1, keepdims=True)

    def gat(Xl, W, a, W2, b2):
        H = Xl @ W
        dh = W.shape[1]
        e = (H @ a[:dh])[:, None] + (H @ a[dh:])[None, :]
        e = np.where(A == 0, np.float32(-1e9), e).astype(np.float32)
        alpha = softmax(e)
        Z = alpha @ H
        zz = Z @ W2 + b2
        return np.where(zz > 0, zz, np.exp(np.minimum(zz, 0)) - 1)

    Z = gat(X, w["W1"], w["a1"], w["W21"], w["b21"])
    Z = gat(Z, A, w["Wg2"], w["a2"], w["W22"], w["b22"])
    g = Z.mean(axis=0)
    h = np.maximum(g @ w["M1"] + w["bm1"], 0)
    return h @ w["M2"] + w["bm2"]


def make_in_maps(X, A, N, n_cores):
    """Host-side sharding: XT bf16 [768, R]; AT fp8 (exact for 0/1) packed
    into the DoubleRow k-pair chunk layout [C2*128, 2, R]."""
    R = N // n_cores
    C2 = N // 256
    Xb = np.asarray(X, np.float32).astype(ml_dtypes.bfloat16)
    Af = np.asarray(A, np.float32).astype(ml_dtypes.float8_e4m3)
    maps = []
    for k in range(n_cores):
        At = Af[k * R:(k + 1) * R, :].T                  # [N, R]
        Ap = np.ascontiguousarray(
            At.reshape(C2, 2, P, R).transpose(0, 2, 1, 3)
        ).reshape(C2 * P, 2, R)
        maps.append({
            "XT": np.ascontiguousarray(Xb[k * R:(k + 1) * R, :].T),
            "AT": Ap,
        })
    return maps


# =====================================================================
# Harness entry point: full inputs in, full output out.
# =====================================================================

_KERNEL_STATE = {}


def kernel(**inputs):
    import numpy as np
    from concourse.bass_utils import run_bass_kernel_spmd

    N = inputs["A"].shape[0]
    n_cores = 8
    w = {k: np.asarray(inputs[k]) for k in
         ("W1", "a1", "W21", "b21", "Wg2", "a2", "W22", "b22",
          "M1", "bm1", "M2", "bm2")}
    nc = build_gat_nc(w, N=N, n_cores=n_cores, reps=1, debug=False)
    nc.finalize()
    in_maps = make_in_maps(np.asarray(inputs["X"]), np.asarray(inputs["A"]),
                           N, n_cores)
    res = run_bass_kernel_spmd(nc, in_maps, core_ids=list(range(n_cores)),
                               trace=False)
    _KERNEL_STATE.update(results=res, nc=nc, in_maps=in_maps, w=w, N=N)
    return np.asarray(res.results[0]["out"]).reshape(2).astype(np.float32)


# revision 60
# speedup vs baseline: 1.3979x; 1.0266x over previous
"""Bass/Tile kernel builder for the 2-layer GAT + MLP-head classifier (v3).

Math trick: with e_ij = s_i + d_j and row-softmax, the s_i term cancels:
  alpha = softmax_j(where(A==0, -1e9, s_i + d_j))
        = A_ij * u_j / sum_j(A_ij * u_j),   u = exp(d)
  Z = alpha @ H = (A @ (u*H)) / (A @ u)
so each GAT layer is one masked matmul A @ [u*H, u] -- the NxN attention
matrix is never materialized.

v3 (fp8 DoubleRow):
- A is 0/1 so fp8e4m3 is exact; host packs each core's A^T shard into the
  DoubleRow k-pair layout [N/256, 128, 2, R] so the PE contracts 256 rows
  per chunk at 2 fp8 MACs/cell/cycle.
- B = [u*H*s, u] quantized to fp8 with a power-of-2 scale s on the uH
  columns (s cancels via 1/s folded into W2); layer-2 needs s2=2^16 to
  clear the fp8 normal range. Verified numerically: rel_err ~7e-3.
- B chunks padded to a 16-byte multiple of columns (80 / 48) to satisfy
  the DoubleRow weight AP step%16 constraint; pad columns are zeroed.
- AllGather payloads are fp8 in the same k-pair chunk layout, so the
  per-chunk stationary loads are single contiguous DMAs.
- A tiles double-buffered across reps (8MB/shard), ag/ar bounce buffers
  parity-alternated, so back-to-back reps pipeline.
"""

import math

import numpy as np
import ml_dtypes

import concourse.bass as bass
import concourse.bacc as bacc
import concourse.mybir as mybir
import concourse.tile as tile

F32 = mybir.dt.float32
F32R = mybir.dt.float32r
BF16 = mybir.dt.bfloat16
FP8 = mybir.dt.float8e4
P = 128

DIN = 768
DH1, DO1 = 64, 64
DH2, DO2 = 32, 64
M1P, M2P = 80, 48            # padded B cols (16B multiple for DoubleRow)
S1, S2 = 8.0, 2.0 ** 16      # fp8 uH column scales, folded into W21/W22


def _bf16(x):
    return np.asarray(x, dtype=np.float32).astype(ml_dtypes.bfloat16)


def _f32(x):
    return np.ascontiguousarray(np.asarray(x, dtype=np.float32))


def _f32r_view(h):
    """Reinterpret an f32 DRAM handle as f32r (same bytes) so the DMA into
    an f32r SBUF tile is a pure byte copy, not a cast."""
    from concourse.bass import DRamTensorHandle
    return DRamTensorHandle(h.name, list(h.shape), F32R)


def _halves(R, step=512):
    out = []
    s = 0
    while s < R:
        e = min(s + step, R)
        out.append((s, e))
        s = e
    return out


def build_gat_nc(weights, N=8192, n_cores=8, reps=1, stage=5, debug=False,
                 no_coll=False, core_id=0):
    R = N // n_cores            # rows per core
    C2 = N // 256               # DoubleRow j-chunks (256-deep contraction)
    RC2 = R // 256              # local 256-row groups per core
    H512 = _halves(R, 512)
    DR = mybir.MatmulPerfMode.DoubleRow

    nc = bacc.Bacc("TRN2", debug=debug, num_devices=n_cores,
                   target_bir_lowering=False)
    groups = [list(range(n_cores))]

    # ---------------- I/O ----------------
    # XT host-packed [P, DIN/P, R] so the whole X^T shard is one DMA with
    # 12KB contiguous per partition; AT host-packed [AG, P, C2/AG, 2, R]
    # (AG=4 groups) so each 2MB group is one DMA with 16KB/partition lines.
    AG = 4
    CPG = C2 // AG              # chunks per A-load group
    xt_d = nc.declare_dram_parameter("XT", [P, DIN // P, R], BF16,
                                     isOutput=False)
    at_d = nc.declare_dram_parameter("AT", [AG * P, CPG, 2, R], FP8,
                                     isOutput=False)
    out_d = nc.declare_dram_parameter("out", [2, 1], F32, isOutput=True)

    # ---------------- constants (inlined into NEFF) ----------------
    w1_d = nc.inline_tensor(_bf16(weights["W1"]), "w1c")           # [768, 64]
    a1d_d = nc.inline_tensor(_bf16(weights["a1"][DH1:]).reshape(DH1, 1), "a1dc")
    w21_d = nc.inline_tensor(_f32(np.asarray(weights["W21"]) / S1), "w21c")
    b21_d = nc.inline_tensor(_f32(weights["b21"]).reshape(DO1, 1), "b21c")
    wg2_d = nc.inline_tensor(_bf16(weights["Wg2"]), "wg2c")        # [64, 32]
    a2d_d = nc.inline_tensor(_bf16(weights["a2"][DH2:]).reshape(DH2, 1), "a2dc")
    w22_d = nc.inline_tensor(_f32(np.asarray(weights["W22"]) / S2), "w22c")
    b22_d = nc.inline_tensor(_f32(weights["b22"]).reshape(DO2, 1), "b22c")
    m1_d = nc.inline_tensor(_f32(weights["M1"]), "m1c")            # [64, 64]
    bm1_d = nc.inline_tensor(_f32(weights["bm1"]).reshape(64, 1), "bm1c")
    m2_d = nc.inline_tensor(_f32(weights["M2"]), "m2c")            # [64, 2]
    bm2_d = nc.inline_tensor(_f32(weights["bm2"]).reshape(2, 1), "bm2c")
    ident_d = nc.inline_tensor(
        np.eye(P, dtype=np.float32).astype(ml_dtypes.bfloat16), "identc")
    ones_d = nc.inline_tensor(np.ones((1, 64), np.float32), "onesc")

    # ---------------- internal DRAM (collective bounce, rep-parity) -------
    ag1_in = [nc.dram_tensor(f"ag1_in{p}", [RC2 * P, 2, M1P], FP8)
              for p in range(2)]
    ag1_out = [nc.dram_tensor(f"ag1_out{p}", [n_cores, RC2, P, 2, M1P], FP8,
                              addr_space="Shared") for p in range(2)]
    ag2_in = [nc.dram_tensor(f"ag2_in{p}", [RC2 * P, 2, M2P], FP8)
              for p in range(2)]
    ag2_out = [nc.dram_tensor(f"ag2_out{p}", [n_cores, RC2, P, 2, M2P], FP8,
                              addr_space="Shared") for p in range(2)]
    ar_in = [nc.dram_tensor(f"ar_in{p}", [64, 1], F32) for p in range(2)]
    ar_out = [nc.dram_tensor(f"ar_out{p}", [n_cores, 64], F32,
                             addr_space="Shared") for p in range(2)]

    with tile.TileContext(nc) as tc:
        with (
            tc.tile_pool(name="const", bufs=1) as cp,
            tc.tile_pool(name="work", bufs=1) as wp,
            tc.tile_pool(name="psum", bufs=1, space="PSUM") as sp_pool,
        ):
            # ---- load constants to SBUF (once)
            w1_sb = cp.tile([P, (DIN // P) * DH1], BF16, name="w1_sb")
            for kc in range(DIN // P):
                nc.sync.dma_start(w1_sb[:, kc * DH1:(kc + 1) * DH1],
                                  w1_d[kc * P:(kc + 1) * P, :])
            a1d_sb = cp.tile([DH1, 1], BF16, name="a1d_sb")
            nc.sync.dma_start(a1d_sb, a1d_d[:])
            a2d_sb = cp.tile([DH2, 1], BF16, name="a2d_sb")
            nc.sync.dma_start(a2d_sb, a2d_d[:])
            w21_sb = cp.tile([DH1, DO1], F32R, name="w21_sb")
            nc.sync.dma_start(w21_sb, _f32r_view(w21_d)[:])
            b21_sb = cp.tile([DO1, 1], F32, name="b21_sb")
            nc.sync.dma_start(b21_sb, b21_d[:])
            wg2_sb = cp.tile([DO1, DH2], BF16, name="wg2_sb")
            nc.sync.dma_start(wg2_sb, wg2_d[:])
            w22_sb = cp.tile([DH2, DO2], F32R, name="w22_sb")
            nc.sync.dma_start(w22_sb, _f32r_view(w22_d)[:])
            b22_sb = cp.tile([DO2, 1], F32, name="b22_sb")
            nc.sync.dma_start(b22_sb, b22_d[:])
            m1_sb = cp.tile([64, 64], F32, name="m1_sb")
            nc.sync.dma_start(m1_sb, m1_d[:])
            bm1_sb = cp.tile([64, 1], F32, name="bm1_sb")
            nc.sync.dma_start(bm1_sb, bm1_d[:])
            m2_sb = cp.tile([64, 2], F32, name="m2_sb")
            nc.sync.dma_start(m2_sb, m2_d[:])
            bm2_sb = cp.tile([2, 1], F32, name="bm2_sb")
            nc.sync.dma_start(bm2_sb, bm2_d[:])
            ident_sb = cp.tile([P, P], BF16, name="ident_sb")
            nc.sync.dma_start(ident_sb, ident_d[:])
            ones_sb = cp.tile([1, 64], F32R, name="ones_sb")
            nc.sync.dma_start(ones_sb, _f32r_view(ones_d)[:])
            lns1_sb = cp.tile([P, 1], F32, name="lns1_sb")
            nc.vector.memset(lns1_sb, float(math.log(S1)))
            lns2_sb = cp.tile([P, 1], F32, name="lns2_sb")
            nc.vector.memset(lns2_sb, float(math.log(S2)))
            ones8_sb = cp.tile([n_cores, 1], F32, name="ones8_sb")
            nc.vector.memset(ones8_sb, 1.0)

            # stage-timing scaffolding: tiny accumulator chain that keeps
            # truncated-stage variants honest (anti-DCE, serializes reps)
            f_acc = cp.tile([1, 1], F32, name="f_acc")
            nc.vector.memset(f_acc, 0.0)

            def consume(src, t):
                g = wp.tile([1, 1], F32, tag="csm", bufs=2, name=f"csm{t}")
                nc.vector.tensor_copy(g, src)
                nc.vector.tensor_tensor(f_acc, g, f_acc,
                                        op=mybir.AluOpType.add)

            def build_own_b(ht_sb, dh, mp, a_sb, lns, ag_in, tag):
                """Write this core's R rows of B = [u*H*s, u, 0pad] (fp8,
                k-pair interleaved chunk layout) to ag_in."""
                for g in range(RC2):
                    bch = wp.tile([P, 2, mp], FP8, tag="bch", bufs=3,
                                  name=f"bch{tag}_{g}")
                    for j in range(2):
                        lc = 2 * g + j
                        hTs = ht_sb[:, lc * P:(lc + 1) * P]
                        ps_tr = sp_pool.tile([P, dh], BF16, tag="sm", bufs=2,
                                             name=f"ps_tr{tag}_{lc}")
                        nc.tensor.transpose(ps_tr, hTs, ident_sb[:dh, :dh])
                        ps_d = sp_pool.tile([P, 1], F32, tag="sm", bufs=2,
                                            name=f"ps_d{tag}_{lc}")
                        nc.tensor.matmul(ps_d, hTs, a_sb, start=True,
                                         stop=True)
                        u = wp.tile([P, 1], F32, tag="u", bufs=4,
                                    name=f"u{tag}_{lc}")
                        nc.scalar.activation(u, ps_d,
                                             mybir.ActivationFunctionType.Exp)
                        us = wp.tile([P, 1], F32, tag="u", bufs=4,
                                     name=f"us{tag}_{lc}")
                        nc.scalar.activation(us, ps_d,
                                             mybir.ActivationFunctionType.Exp,
                                             bias=lns)
                        nc.vector.tensor_scalar_mul(bch[:, j, 0:dh], ps_tr, us)
                        nc.vector.tensor_copy(bch[:, j, dh:dh + 1], u)
                        nc.vector.memset(bch[:, j, dh + 1:mp], 0.0)
                    nc.sync.dma_start(ag_in[g * P:(g + 1) * P, :, :], bch)

            def masked_accum(ps_o, ag_out, mp, abf, tag):
                """ps_o[mp, R] += B_c^T @ A_c over all 256-deep chunks.
                The whole gathered B comes back in ONE DMA (640B+ lines)."""
                bT = wp.tile([P, n_cores, RC2, 2, mp], FP8,
                             tag=f"bT{mp}", bufs=1, name=f"bT{tag}")
                nc.sync.dma_start(bT, ag_out[:].transpose([2, 0, 1, 3, 4]))
                for c in range(C2):
                    k, g = c // RC2, c % RC2
                    for h, (s, e) in enumerate(H512):
                        nc.tensor.matmul(ps_o[:, s:e], bT[:, k, g, :, :],
                                         abf[c // CPG][:, c % CPG, :, s:e],
                                         start=(c == 0), stop=(c == C2 - 1),
                                         perf_mode=DR)

            def epilogue(ps_o, dh, do, w2_sb, b2_sb, out_dt, tag):
                """out_t = elu((numer/denom) @ W2 + b2), transposed [do, R].

                f32r matmuls (1 cyc/row at free>=256, ~10-bit mantissa):
                broadcast the raw denominator to [do, R], reciprocal at full
                partition width, multiply into the W2 product."""
                o_t = wp.tile([dh, R], F32R, tag="o_t", bufs=1,
                              name=f"o_t{tag}")
                nc.scalar.copy(o_t, ps_o[0:dh, :])
                den = wp.tile([1, R], F32R, tag="den", bufs=1,
                              name=f"den{tag}")
                nc.scalar.copy(den, ps_o[dh:dh + 1, :])
                bc_sb = wp.tile([do, R], F32, tag="bc", bufs=1,
                                name=f"bc{tag}")
                zt = wp.tile([do, R], F32, tag="ep", bufs=4, name=f"zt{tag}")
                for h, (s, e) in enumerate(H512):
                    ps_bc = sp_pool.tile([do, e - s], F32, tag="sm", bufs=2,
                                         name=f"ps_bc{tag}_{h}")
                    nc.tensor.matmul(ps_bc, ones_sb[:, :do], den[:, s:e],
                                     start=True, stop=True)
                    nc.vector.reciprocal(bc_sb[:, s:e], ps_bc)
                    ps_p = sp_pool.tile([do, e - s], F32, tag="sm", bufs=2,
                                        name=f"ps_p{tag}_{h}")
                    nc.tensor.matmul(ps_p, w2_sb, o_t[0:dh, s:e],
                                     start=True, stop=True)
                    nc.vector.tensor_tensor(zt[:, s:e], ps_p, bc_sb[:, s:e],
                                            op=mybir.AluOpType.mult)
                # ELU: r = relu(zt+b2); en = exp(min(zt+b2, 0)); out = r+en-1
                nm = wp.tile([do, R], F32, tag="ep", bufs=4, name=f"nm{tag}")
                nc.vector.tensor_scalar(nm, zt, b2_sb, 0.0,
                                        op0=mybir.AluOpType.add,
                                        op1=mybir.AluOpType.min)
                en = wp.tile([do, R], F32, tag="ep", bufs=4, name=f"en{tag}")
                nc.scalar.activation(en, nm, mybir.ActivationFunctionType.Exp)
                r = wp.tile([do, R], F32, tag="ep", bufs=4, name=f"r{tag}")
                nc.scalar.activation(r, zt, mybir.ActivationFunctionType.Relu,
                                     bias=b2_sb)
                nb = 2 if out_dt == BF16 else 1
                out_t = wp.tile([do, R], out_dt, tag=f"out_t{out_dt}",
                                bufs=nb, name=f"out_t{tag}")
                nc.vector.scalar_tensor_tensor(
                    out_t, in0=r, scalar=-1.0, in1=en,
                    op0=mybir.AluOpType.add, op1=mybir.AluOpType.add)
                return out_t

            # ============================================================
            # Phase functions (i = rep index). State shared via parity dicts.
            # ============================================================
            abf_gen = {}   # parity -> list of 32 A-chunk tiles
            xt_gen = {}    # parity -> list of 6 XT tiles
            ag_coll = (lambda ag_i, ag_o: (
                nc.sync.dma_start(ag_o[0:RC2 * P, :, :], ag_i[:])
                if no_coll else nc.gpsimd.collective_compute(
                    "AllGather", mybir.AluOpType.bypass,
                    replica_groups=groups, ins=[ag_i[:]], outs=[ag_o[:]])))

            def phase_Xpre(i):
                """Prefetch rep-i XT (one 1.5MB DMA), >=1 iteration early."""
                if 0 <= i < reps:
                    xt_gen[i % 2] = xts = wp.tile(
                        [P, DIN // P, R], BF16, tag="xt", bufs=2,
                        name=f"xtr{i}")
                    nc.sync.dma_start(xts, xt_d[:])

            def phase_Fdma(i):
                """Issue rep-i A-shard loads (4 x 2MB)."""
                rr = f"r{i}"
                abf_gen[i % 2] = abf = [
                    wp.tile([P, CPG, 2, R], FP8, tag="abf", bufs=2 * AG,
                            name=f"abf{rr}_{g}") for g in range(AG)]
                for g in range(AG):
                    nc.sync.dma_start(abf[g],
                                      at_d[g * P:(g + 1) * P, :, :, :])
                return abf

            def phase_Frest(i):
                """H1 (from prefetched XT), B1 build, AllGather-1 issue."""
                rr = f"r{i}"
                xts = xt_gen[i % 2]
                ps_h1 = sp_pool.tile([DH1, R], F32, tag="big", bufs=3,
                                     name=f"ps_h1{rr}")
                for kc in range(DIN // P):
                    for h, (s, e) in enumerate(H512):
                        nc.tensor.matmul(ps_h1[:, s:e],
                                         w1_sb[:, kc * DH1:(kc + 1) * DH1],
                                         xts[:, kc, s:e],
                                         start=(kc == 0),
                                         stop=(kc == DIN // P - 1))
                h1t_sb = wp.tile([DH1, R], BF16, tag="ht", bufs=1,
                                 name=f"h1t{rr}")
                nc.scalar.copy(h1t_sb, ps_h1)
                build_own_b(h1t_sb, DH1, M1P, a1d_sb, lns1_sb,
                            ag1_in[i % 2], "b1" + rr)
                ag_coll(ag1_in[i % 2], ag1_out[i % 2])

            def phase_Ma(i):
                """Masked matmul layer 1 + epilogue 1 -> out1t."""
                rr = f"r{i}"
                ps_o1 = sp_pool.tile([M1P, R], F32, tag="big", bufs=3,
                                     name=f"ps_o1{rr}")
                masked_accum(ps_o1, ag1_out[i % 2], M1P, abf_gen[i % 2],
                             "1" + rr)
                return epilogue(ps_o1, DH1, DO1, w21_sb, b21_sb, BF16,
                                "1" + rr)

            def phase_Mb(i, out1t):
                """H2, B2 build, AllGather-2 issue."""
                rr = f"r{i}"
                ps_h2 = sp_pool.tile([DH2, R], F32, tag="big", bufs=3,
                                     name=f"ps_h2{rr}")
                for h, (s, e) in enumerate(H512):
                    nc.tensor.matmul(ps_h2[:, s:e], wg2_sb, out1t[:, s:e],
                                     start=True, stop=True)
                h2t_sb = wp.tile([DH2, R], BF16, tag="ht2", bufs=1,
                                 name=f"h2t{rr}")
                nc.scalar.copy(h2t_sb, ps_h2)
                build_own_b(h2t_sb, DH2, M2P, a2d_sb, lns2_sb,
                            ag2_in[i % 2], "b2" + rr)
                ag_coll(ag2_in[i % 2], ag2_out[i % 2])

            def phase_Ta(i):
                """Masked matmul layer 2 + epilogue 2 -> out2t."""
                rr = f"r{i}"
                ps_o2 = sp_pool.tile([M2P, R], F32, tag="big", bufs=3,
                                     name=f"ps_o2{rr}")
                masked_accum(ps_o2, ag2_out[i % 2], M2P, abf_gen[i % 2],
                             "2" + rr)
                return epilogue(ps_o2, DH2, DO2, w22_sb, b22_sb, F32,
                                "2" + rr)

            def phase_Tb(i, out2t):
                """Partial mean + head AllGather issue."""
                rr = f"r{i}"
                gsum = wp.tile([64, 1], F32, tag="gsum", bufs=8,
                               name=f"gsum{rr}")
                nc.vector.reduce_sum(gsum, out2t, axis=mybir.AxisListType.X)
                nc.sync.dma_start(ar_in[i % 2][:], gsum)
                if no_coll:
                    nc.sync.dma_start(ar_out[i % 2][0:1, :], ar_in[i % 2][:])
                else:
                    nc.gpsimd.collective_compute(
                        "AllGather", mybir.AluOpType.bypass,
                        replica_groups=groups,
                        ins=[ar_in[i % 2][:]], outs=[ar_out[i % 2][:]])

            def phase_H(i):
                """Cross-core sum (g8^T @ ones) + MLP head + out."""
                rr = f"r{i}"
                g8 = wp.tile([n_cores, 64], F32, tag="gsum", bufs=8,
                             name=f"g8{rr}")
                nc.sync.dma_start(g8, ar_out[i % 2][:])
                ps_g = sp_pool.tile([64, 1], F32, tag="sm", bufs=2,
                                    name=f"ps_g{rr}")
                nc.tensor.matmul(ps_g, g8, ones8_sb, start=True, stop=True)
                g_sb = wp.tile([64, 1], F32, tag="gsum", bufs=8,
                               name=f"g{rr}")
                nc.scalar.copy(g_sb, ps_g)
                ps_hd = sp_pool.tile([64, 1], F32, tag="sm", bufs=2,
                                     name=f"ps_hd{rr}")
                nc.tensor.matmul(ps_hd, m1_sb, g_sb, start=True, stop=True)
                h_sb = wp.tile([64, 1], F32, tag="gsum", bufs=8,
                               name=f"h{rr}")
                nc.scalar.activation(h_sb, ps_hd,
                                     mybir.ActivationFunctionType.Relu,
                                     bias=bm1_sb, scale=1.0 / N)
                ps_f = sp_pool.tile([2, 1], F32, tag="sm", bufs=2,
                                    name=f"ps_f{rr}")
                nc.tensor.matmul(ps_f, m2_sb, h_sb, start=True, stop=True)
                f_sb = wp.tile([2, 1], F32, tag="gsum", bufs=8,
                               name=f"f{rr}")
                nc.scalar.activation(f_sb, ps_f,
                                     mybir.ActivationFunctionType.Identity,
                                     bias=bm2_sb)
                nc.sync.dma_start(out_d[:], f_sb)

            if stage >= 5:
                # Software-pipelined schedule. Iteration i runs
                #   XTpre(i+1) | Frest(i){H1,B1,AG1} | Ma/Mb(i-1) |
                #   Ta/Tb(i-2) | H(i-3) | Fdma(i){A}
                # so every AllGather gets ~a full iteration of other reps'
                # PE work between issue and its consumer hitting the
                # in-order PE queue.
                phase_Xpre(0)
                for i in range(reps + 3):
                    phase_Xpre(i + 1)
                    if i < reps:
                        phase_Frest(i)
                    if 1 <= i <= reps:
                        phase_Mb(i - 1, phase_Ma(i - 1))
                    if 2 <= i <= reps + 1:
                        phase_Tb(i - 2, phase_Ta(i - 2))
                    if 3 <= i <= reps + 2:
                        phase_H(i - 3)
                    if i < reps:
                        phase_Fdma(i)
            else:
                # sequential fallback with stage gates (attribution builds)
                for rep in range(reps):
                    rr = f"r{rep}"
                    xt_gen[rep % 2] = xts = wp.tile(
                        [P, DIN // P, R], BF16, tag="xt", bufs=2,
                        name=f"xt{rr}")
                    nc.sync.dma_start(xts, xt_d[:])
                    abf = phase_Fdma(rep, prefetch_xt=False)
                    if stage <= 1:
                        consume(abf[AG - 1][0:1, 0:1, 0:1, 0:1], "s1" + rr)
                        continue
                    phase_Frest(rep)
                    if stage <= 2:
                        gt = wp.tile([1, 1], FP8, tag="gt", bufs=2,
                                     name=f"gt{rr}")
                        nc.sync.dma_start(
                            gt, ag1_out[rep % 2][n_cores - 1:n_cores,
                                                 RC2 - 1:RC2, P - 1:P,
                                                 1:2, 0:1])
                        consume(gt, "s2" + rr)
                        continue
                    out1t = phase_Ma(rep)
                    if stage <= 3:
                        consume(out1t[0:1, 0:1], "s3" + rr)
                        continue
                    phase_Mb(rep, out1t)
                    out2t = phase_Ta(rep)
                    if stage <= 4:
                        consume(out2t[0:1, 0:1], "s4" + rr)
                        continue
                    phase_Tb(rep, out2t)
                    phase_H(rep)

            if stage <= 4:
                nc.sync.dma_start(out_d[0:1, :], f_acc)

    return nc


def numpy_reference(X, A, w):
    """Straight fp32 numpy port of the jax reference (for small-N checks)."""
    def softmax(e):
        m = e.max(axis=1, keepdims=True)
        x = np.exp(e - m)
        return x / x.sum(axis=1, keepdims=True)

    def gat(Xl, W, a, W2, b2):
        H = Xl @ W
        dh = W.shape[1]
        e = (H @ a[:dh])[:, None] + (H @ a[dh:])[None, :]
        e = np.where(A == 0, np.float32(-1e9), e).astype(np.float32)
        alpha = softmax(e)
        Z = alpha @ H
        zz = Z @ W2 + b2
        return np.where(zz > 0, zz, np.exp(np.minimum(zz, 0)) - 1)

    Z = gat(X, w["W1"], w["a1"], w["W21"], w["b21"])
    Z = gat(Z, A, w["Wg2"], w["a2"], w["W22"], w["b22"])
    g = Z.mean(axis=0)
    h = np.maximum(g @ w["M1"] + w["bm1"], 0)
    return h @ w["M2"] + w["bm2"]


def make_in_maps(X, A, N, n_cores):
    """Host-side sharding: XT bf16 packed [128, DIN/128, R] (one DMA, 12KB
    contiguous per partition); AT fp8 (exact for 0/1) packed into the
    DoubleRow k-pair layout grouped for 2MB DMAs:
    [AG*128, C2/AG, 2, R] with 16KB contiguous per partition per group."""
    R = N // n_cores
    C2 = N // 256
    AG = 4
    CPG = C2 // AG
    Xb = np.asarray(X, np.float32).astype(ml_dtypes.bfloat16)
    Af = np.asarray(A, np.float32).astype(ml_dtypes.float8_e4m3)
    maps = []
    for k in range(n_cores):
        At = Af[k * R:(k + 1) * R, :].T                  # [N, R]
        Ap = np.ascontiguousarray(
            At.reshape(AG, CPG, 2, P, R).transpose(0, 3, 1, 2, 4)
        ).reshape(AG * P, CPG, 2, R)
        Xp = np.ascontiguousarray(
            Xb[k * R:(k + 1) * R, :].T.reshape(DIN // P, P, R)
            .transpose(1, 0, 2))
        maps.append({"XT": Xp, "AT": Ap})
    return maps


# =====================================================================
# Harness entry point: full inputs in, full output out.
# =====================================================================

_KERNEL_STATE = {}


def kernel(**inputs):
    import numpy as np
    from concourse.bass_utils import run_bass_kernel_spmd

    N = inputs["A"].shape[0]
    n_cores = 8
    w = {k: np.asarray(inputs[k]) for k in
         ("W1", "a1", "W21", "b21", "Wg2", "a2", "W22", "b22",
          "M1", "bm1", "M2", "bm2")}
    nc = build_gat_nc(w, N=N, n_cores=n_cores, reps=1, debug=False)
    nc.finalize()
    in_maps = make_in_maps(np.asarray(inputs["X"]), np.asarray(inputs["A"]),
                           N, n_cores)
    res = run_bass_kernel_spmd(nc, in_maps, core_ids=list(range(n_cores)),
                               trace=False)
    _KERNEL_STATE.update(results=res, nc=nc, in_maps=in_maps, w=w, N=N)
    return np.asarray(res.results[0]["out"]).reshape(2).astype(np.float32)
